# revision 14
# baseline (speedup 1.0000x reference)
"""MoE gate (softmax router + top-2 + aux loss) on 8 Trainium2 NeuronCores.

Strategy (data-parallel, per the sharding hint):
  - Flatten x to (16384, 2048) tokens and shard 2048 tokens per core.
  - Host-side: pass each shard TRANSPOSED (xt = shard.T, shape [D, TOK]) so the
    contraction dim D lands on SBUF partitions; replicate W.T per core.
  - Device: scores = xt_tile.T @ W.T accumulated over 16 d-chunks in PSUM,
    softmax over the 8 experts (free dim), hardware max8/max_index sort for
    top-2 values+indices; probs + top2(vals,idx) packed into one output.
  - Host-side: gather shards, compute the (tiny) aux-loss reduction from
    per-token probs and indices.
"""

import sys

for _p in ("/opt/trn_rl_repo", "/root/.axon_site/_ro/trn_rl_repo"):
    if _p not in sys.path:
        sys.path.append(_p)

from contextlib import ExitStack

import numpy as np

import concourse.bass as bass
import concourse.mybir as mybir
import concourse.tile as tile
from concourse.bass_utils import run_bass_kernel_spmd
from concourse.tile_rust import add_dep_helper

N_CORES = 8
B, N, D = 4, 4096, 2048
E = 8          # experts
K = 2          # top-k
TOK_TOTAL = B * N
TOK = TOK_TOTAL // N_CORES   # tokens per core = 2048
P = 128
DC = D // P                  # d-chunks = 16
TG = TOK // P                # token groups per core = 16
OUTC = E + 2 * K             # packed output row: 8 probs, 2 weights, 2 idx
ALPHA = 0.1
ROUTE_SCALE = 1.0
F32 = mybir.dt.float32


def build_bass() -> bass.Bass:
    nc = bass.Bass()
    xt = nc.declare_dram_parameter("xt", [D, TOK], F32, isOutput=False)
    wt = nc.declare_dram_parameter("wt", [DC, P, E], F32, isOutput=False)
    # Packed per-token row: [probs(8) | top2 weights(2) | top2 idx bits(2)]
    out_all = nc.declare_dram_parameter("out_all", [TOK, OUTC], F32, isOutput=True)

    with tile.TileContext(nc) as tc, ExitStack() as ctx:
        xt_pool = ctx.enter_context(tc.tile_pool(name="xt", bufs=DC))
        const_pool = ctx.enter_context(tc.tile_pool(name="const", bufs=1))
        work = ctx.enter_context(tc.tile_pool(name="work", bufs=1))
        psum = ctx.enter_context(tc.tile_pool(name="psum", bufs=8, space="PSUM"))

        # W.T resident in SBUF: [128 (d within chunk), chunk, expert]
        wt_sb = const_pool.tile([P, DC, E], F32)
        hw_dmas = [nc.sync.dma_start(wt_sb[:], wt.rearrange("c p e -> p c e"))]

        # Dummy matmul consuming wt_sb: absorbs the wt-DMA wait on PE so real
        # matmuls carry at most one sync wait (the fp32 self-loading LDW
        # struct only has a single wait slot) — and warms the PE HAM.
        ps_warm = psum.tile([P, E], F32, tag="ps", name="ps_warm")
        nc.tensor.matmul(
            ps_warm[0:E, :], lhsT=wt_sb[:, 0, :], rhs=wt_sb[:, 0, :],
            start=True, stop=True,
        )

        # All 16 x-slabs stay resident (128 KiB/partition); one 1 MiB DMA each.
        xt_tiles = []
        for d in range(DC):
            xt_sb = xt_pool.tile([P, TOK], F32, tag="xt_slab")
            hw_dmas.append(nc.sync.dma_start(xt_sb[:], xt[bass.ts(d, P), :]))
            xt_tiles.append(xt_sb)

        # Softmax over the 8 experts. Logits are O(1) so exp() without the
        # max-subtraction is safe in f32 and mathematically identical.
        # Token-group scores accumulate in PSUM; max 8 open accumulation
        # groups (one per PSUM bank), so process token groups in two batches,
        # d-outer inside so PE consumes slabs as the DMAs land.
        exp_sb = work.tile([P, TG, E], F32)
        mm_insts = []
        act_insts = []
        for half in range(2):
            ps_tiles = [
                psum.tile([P, E], F32, tag="ps", name=f"ps_{half}_{tl}")
                for tl in range(TG // 2)
            ]
            for d in range(DC):
                for tl in range(TG // 2):
                    t = half * (TG // 2) + tl
                    mm_insts.append(nc.tensor.matmul(
                        ps_tiles[tl][:],
                        lhsT=xt_tiles[d][:, bass.ts(t, P)],
                        rhs=wt_sb[:, d, :],
                        start=(d == 0),
                        stop=(d == DC - 1),
                    ))
            for tl in range(TG // 2):
                t = half * (TG // 2) + tl
                act_insts.append(nc.scalar.activation(
                    exp_sb[:, t, :], ps_tiles[tl][:],
                    mybir.ActivationFunctionType.Exp,
                ))

        s_sb = work.tile([P, TG, 1], F32)
        nc.vector.reduce_sum(s_sb[:], exp_sb[:], axis=mybir.AxisListType.X)
        r_sb = work.tile([P, TG, 1], F32)
        nc.vector.reciprocal(r_sb[:], s_sb[:])

        # Packed output tile: [.., g, 0:8]=probs, [.., g, 8:10]=w, [.., 10:12]=idx
        out_sb = work.tile([P, TG, OUTC], F32)
        probs_view = out_sb[:, :, 0:E]
        nc.vector.tensor_tensor(
            probs_view, exp_sb[:], r_sb[:].to_broadcast([P, TG, E]),
            op=mybir.AluOpType.mult,
        )

        # Hardware top-8 sort per token: values descending + their indices.
        m8 = work.tile([P, TG, E], F32)
        i8 = work.tile([P, TG, E], mybir.dt.uint32)
        dve_insts = []
        for t in range(TG):
            dve_insts.append(nc.vector.max(out=m8[:, t, :], in_=out_sb[:, t, 0:E]))
            dve_insts.append(nc.vector.max_index(
                out=i8[:, t, :], in_max=m8[:, t, :], in_values=out_sb[:, t, 0:E]
            ))
        dve_insts.append(
            nc.vector.tensor_copy(out_sb[:, :, E : E + K], m8[:, :, 0:K])
        )
        dve_insts.append(nc.vector.tensor_copy(
            out_sb[:, :, E + K : OUTC].bitcast(mybir.dt.uint32), i8[:, :, 0:K]
        ))

        # Token tg*128+p lives at partition p, group tg. One output DMA.
        out_dma = nc.gpsimd.dma_start(
            out_all.rearrange("(g p) c -> p g c", p=P), out_sb[:]
        )

        # Pre-drain absorb chain: every instruction encoding has a single
        # sync-wait slot (EventSemaphore: 2), but the auto-emitted kernel-tail
        # drain gets one wait per outstanding sem lane — which walrus rejects.
        # Emit one SP nop per producer group (one sem lane each) so SP's
        # engine clock observes everything and the drain needs no waits.
        wait_groups = [[d] for d in hw_dmas]
        wait_groups += [act_insts, mm_insts, dve_insts, [out_dma]]
        prev = None
        for gi, group in enumerate(wait_groups):
            nop = nc.sync.nop(hint=f"tail_absorb_{gi}")
            if prev is None:
                # Keep the nop chain after every HWDGE issue so SP never
                # stalls a pending input DMA behind a completion wait.
                for d in hw_dmas:
                    add_dep_helper(nop.ins, d.ins, sync=False,
                                   reason="tail after DMA issue")
            else:
                add_dep_helper(nop.ins, prev.ins, sync=False, reason="tail chain")
            for producer in group:
                add_dep_helper(nop.ins, producer.ins, sync=True,
                               reason="pre-drain absorb")
            prev = nop

    return nc


def make_in_maps(x: np.ndarray, W: np.ndarray) -> list[dict[str, np.ndarray]]:
    xf = np.ascontiguousarray(x, dtype=np.float32).reshape(TOK_TOTAL, D)
    wt_np = np.ascontiguousarray(W.T.astype(np.float32)).reshape(DC, P, E)
    in_maps = []
    for c in range(N_CORES):
        shard = xf[c * TOK : (c + 1) * TOK]
        in_maps.append({"xt": np.ascontiguousarray(shard.T), "wt": wt_np})
    return in_maps


def _ensure_ntff_hook():
    """The agent image's antenv lacks axon_hooks; shim it so trace=True works."""
    try:
        from antenv import axon_hooks  # noqa: F401
        return
    except ImportError:
        pass
    import types

    import antenv

    mod = types.ModuleType("antenv.axon_hooks")
    state = {"hook": None}
    mod.set_axon_ntff_profile_hook = lambda h: state.__setitem__("hook", h)
    mod.get_axon_ntff_profile_hook = lambda: state["hook"]
    sys.modules["antenv.axon_hooks"] = mod
    antenv.axon_hooks = mod
    try:
        from trn_agent_boot.trn_boot import _ntff_profile_via_ctypes

        mod.set_axon_ntff_profile_hook(
            _ntff_profile_via_ctypes("/opt/axon/libaxon_pjrt.so")
        )
    except Exception:
        pass


def run_full(inputs: dict, trace: bool = False):
    if trace:
        _ensure_ntff_hook()
    x = np.asarray(inputs["x"])
    W = np.asarray(inputs["W"])
    in_maps = make_in_maps(x, W)
    nc = build_bass()
    br = run_bass_kernel_spmd(nc, in_maps, list(range(N_CORES)), trace=trace)
    res = br.results
    packed = np.concatenate([res[c]["out_all"] for c in range(N_CORES)], axis=0)
    probs = packed[:, 0:E]
    weight = np.ascontiguousarray(packed[:, E : E + K]) * ROUTE_SCALE
    idx = np.ascontiguousarray(packed[:, E + K : OUTC]).view(np.int32)

    # aux load-balancing loss (tiny reduction, done on host)
    pi = probs.reshape(B, N, E).astype(np.float64).mean(axis=1)          # (B, E)
    counts = np.zeros((B, E), dtype=np.float64)
    idx_b = idx.reshape(B, N * K)
    for b in range(B):
        counts[b] = np.bincount(idx_b[b].ravel(), minlength=E)[:E]
    fi = counts * (E / (K * N))
    aux_loss = np.float32((fi * pi).sum(axis=1).mean() * ALPHA)

    return (weight.astype(np.float32), idx, aux_loss), br


def kernel(**inputs):
    (weight, idx, aux_loss), _ = run_full(inputs, trace=False)
    return weight, idx, aux_loss
